# revision 2
# baseline (speedup 1.0000x reference)
"""MultiHeadAttention (RoPE, causal+padding masks, score-averaging with prev)
on 8 Trainium2 NeuronCores.

Sharding: batch*heads across cores — core i handles b = i//4 and heads
[4*(i%4) .. 4*(i%4)+3]. Projection weights are column/row-sliced per core
(tensor parallel over heads); the output projection produces per-core
partials summed on the host during the gather.

Device pipeline per core (all fp32):
  qkv projections (transposed q/k layout, natural v) -> RoPE via a
  pair-permutation matmul -> per-head-pair scores with row-tiled K=64
  matmul pairs -> additive mask (-inf) -> masked scores to DRAM
  (prev_new[1]) -> +prev, exp(0.5*x) with accumulated row sums ->
  normalize -> PE-transpose of the weight tiles -> attention x V with
  col-tiled pairs -> output projection (transposed), host-summed.
"""
import os
import sys
import types

import numpy as np

try:
    import concourse.bass as bass
except ImportError:
    sys.path.insert(0, "/opt/trn_rl_repo")
    import concourse.bass as bass

import concourse.mybir as mybir
import concourse.tile as tile
from concourse.bass import ts
from concourse.bass_utils import run_bass_kernel_spmd
from contextlib import ExitStack

F32 = mybir.dt.float32
AF = mybir.ActivationFunctionType
ALU = mybir.AluOpType

N_HEADS = 16
D_MODEL = 1024
B = 2
S = 1024
D_K = 64          # head dim
HPC = 4           # heads per core
N_CORES = 8
NEG_INF = float("-inf")

LAST_EXEC_TIME_NS = None


# ---------------------------------------------------------------- wait split
def _split_multi_waits(nc, max_waits=1):
    """This walrus build takes at most one semaphore wait per instruction;
    hoist extras onto NoOps just before it on the same engine stream."""
    n = 0
    for f in nc.m.functions:
        for bb in f.blocks:
            insts = bb.instructions
            if not any(
                i.sync_info and i.sync_info.on_wait
                and len(i.sync_info.on_wait) > max_waits
                for i in insts
            ):
                continue
            new = []
            for inst in insts:
                si = inst.sync_info
                if si is not None and si.on_wait and len(si.on_wait) > max_waits:
                    waits = list(si.on_wait)
                    for w in waits[:-max_waits]:
                        n += 1
                        new.append(mybir.InstNoOp(
                            name=f"{inst.name}-wsplit{n}",
                            engine=inst.engine, ins=[], outs=[],
                            sync_info=mybir.SyncInfo(on_wait=[w], on_update=[]),
                        ))
                    inst.sync_info = mybir.SyncInfo(
                        on_wait=waits[-max_waits:], on_update=list(si.on_update))
                new.append(inst)
            bb.instructions = new
    return n


# ---------------------------------------------------------------- ntff hook
def _install_ntff_hook():
    """Wire up the axon NTFF profile hook missing from this image's antenv
    so trace=True works (used by test harnesses via BASS_TRACE=1)."""
    if "antenv.axon_hooks" in sys.modules:
        return
    try:
        if "/root/.axon_site" not in sys.path:
            sys.path.insert(0, "/root/.axon_site")
        from trn_agent_boot import trn_boot
        hook = trn_boot._ntff_profile_via_ctypes("/opt/axon/libaxon_pjrt.so")
        mod = types.ModuleType("antenv.axon_hooks")
        holder = [hook]
        mod.get_axon_ntff_profile_hook = lambda: holder[0]
        mod.set_axon_ntff_profile_hook = lambda h: holder.__setitem__(0, h)
        sys.modules["antenv.axon_hooks"] = mod
        import antenv
        antenv.axon_hooks = mod
    except Exception:
        pass


# ---------------------------------------------------------------- program
_PROGRAM = None


def _build_program():
    nc = bass.Bass()

    # inputs (per core)
    xqT = nc.dram_tensor("xqT", [D_MODEL, S], F32, kind="ExternalInput")
    xkT = nc.dram_tensor("xkT", [D_MODEL, S], F32, kind="ExternalInput")
    wqT = nc.dram_tensor("wqT", [D_MODEL, 256], F32, kind="ExternalInput")
    wkT = nc.dram_tensor("wkT", [D_MODEL, 256], F32, kind="ExternalInput")
    wvT = nc.dram_tensor("wvT", [D_MODEL, 256], F32, kind="ExternalInput")
    wpT = nc.dram_tensor("wpT", [256, D_MODEL], F32, kind="ExternalInput")
    prevh = nc.dram_tensor("prevh", [HPC, S, S], F32, kind="ExternalInput")
    maskn = nc.dram_tensor("maskn", [S, S], F32, kind="ExternalInput")
    costab = nc.dram_tensor("costab", [128, S], F32, kind="ExternalInput")
    sintab = nc.dram_tensor("sintab", [128, S], F32, kind="ExternalInput")
    s2 = nc.dram_tensor("s2", [128, D_K], F32, kind="ExternalInput")
    idn = nc.dram_tensor("idn", [128, 128], F32, kind="ExternalInput")

    # outputs
    scr = nc.dram_tensor("scr", [HPC, S, S], F32, kind="ExternalOutput")
    outp = nc.dram_tensor("outp", [D_MODEL, S], F32, kind="ExternalOutput")

    with tile.TileContext(nc) as tc, ExitStack() as top:
        # long-lived sbuf
        consts = top.enter_context(tc.tile_pool(name="consts", bufs=1))
        qk_pool = top.enter_context(tc.tile_pool(name="qk", bufs=1))
        v_pool = top.enter_context(tc.tile_pool(name="vp", bufs=1))
        mask_pool = top.enter_context(tc.tile_pool(name="maskp", bufs=1))
        at_pool = top.enter_context(tc.tile_pool(name="atall", bufs=1))

        idn_t = consts.tile([128, 128], F32, tag="idn", name="idn")
        nc.sync.dma_start(idn_t[:], idn[:])
        s2_t = consts.tile([128, D_K], F32, tag="s2", name="s2")
        nc.sync.dma_start(s2_t[:], s2[:])
        cos_t = consts.tile([128, S], F32, tag="cos", name="cos")
        nc.sync.dma_start(cos_t[:], costab[:])
        sin_t = consts.tile([128, S], F32, tag="sin", name="sin")
        nc.sync.dma_start(sin_t[:], sintab[:])
        wp_t = [consts.tile([128, D_MODEL], F32, tag=f"wp{i}", name=f"wp{i}") for i in range(2)]
        for i in range(2):
            nc.sync.dma_start(wp_t[i][:], wpT[ts(i, 128), :])

        mask_t = [mask_pool.tile([128, S], F32, tag=f"mask{qi}", name=f"mask{qi}") for qi in range(8)]
        for qi in range(8):
            nc.sync.dma_start(mask_t[qi][:], maskn[ts(qi, 128), :])

        # rope'd q/k pair tiles (partitions: head 2t ch 0-63 | head 2t+1)
        qr = [qk_pool.tile([128, S], F32, tag=f"qr{t}", name=f"qr{t}") for t in range(2)]
        kr = [qk_pool.tile([128, S], F32, tag=f"kr{t}", name=f"kr{t}") for t in range(2)]
        v_t = [v_pool.tile([128, 256], F32, tag=f"v{st}", name=f"v{st}") for st in range(8)]
        at_all = [at_pool.tile([128, S], F32, tag=f"at{t}", name=f"at{t}") for t in range(2)]

        # ---------------- phase 1+2: loads + qkv projections
        with ExitStack() as ph:
            xin = ph.enter_context(tc.tile_pool(name="xin", bufs=1))
            win = ph.enter_context(tc.tile_pool(name="win", bufs=1))
            pjp = ph.enter_context(tc.tile_pool(name="pjp", bufs=2, space="PSUM"))
            pvp = ph.enter_context(tc.tile_pool(name="pvp", bufs=2, space="PSUM"))
            rotp = ph.enter_context(tc.tile_pool(name="rotp", bufs=2, space="PSUM"))
            rtmp = ph.enter_context(tc.tile_pool(name="rtmp", bufs=2))
            qraw_pool = ph.enter_context(tc.tile_pool(name="qraw", bufs=1))

            xq_t = [xin.tile([128, S], F32, tag=f"xq{i}", name=f"xq{i}") for i in range(8)]
            xk_t = [xin.tile([128, S], F32, tag=f"xk{i}", name=f"xk{i}") for i in range(8)]
            for i in range(8):
                nc.sync.dma_start(xq_t[i][:], xqT[ts(i, 128), :])
                nc.sync.dma_start(xk_t[i][:], xkT[ts(i, 128), :])
            wq_t = [win.tile([128, 256], F32, tag=f"wq{i}", name=f"wq{i}") for i in range(8)]
            wk_t = [win.tile([128, 256], F32, tag=f"wk{i}", name=f"wk{i}") for i in range(8)]
            wv_t = [win.tile([128, 256], F32, tag=f"wv{i}", name=f"wv{i}") for i in range(8)]
            for i in range(8):
                nc.sync.dma_start(wq_t[i][:], wqT[ts(i, 128), :])
                nc.sync.dma_start(wk_t[i][:], wkT[ts(i, 128), :])
                nc.sync.dma_start(wv_t[i][:], wvT[ts(i, 128), :])

            qraw = [qraw_pool.tile([128, S], F32, tag=f"qq{t}", name=f"qq{t}") for t in range(2)]
            kraw = [qraw_pool.tile([128, S], F32, tag=f"kk{t}", name=f"kk{t}") for t in range(2)]

            # qT/kT: psum[p, s-chunk] = sum_d w*T[d, 128t+p] * x*T[d, s]
            for dst, wtiles, xtiles in ((qraw, wq_t, xq_t), (kraw, wk_t, xk_t)):
                for t in range(2):
                    for ch in range(2):
                        pj = pjp.tile([128, 512], F32, tag="pj", name="pj")
                        for i in range(8):
                            nc.tensor.matmul(
                                pj[:], wtiles[i][:, ts(t, 128)],
                                xtiles[i][:, ts(ch, 512)],
                                start=(i == 0), stop=(i == 7))
                        nc.scalar.copy(dst[t][:, ts(ch, 512)], pj[:])
            # v natural: psum[s-tile, 256] = sum_d xkT[d, s]^T... stationary
            for st in range(8):
                pv = pvp.tile([128, 256], F32, tag="pv", name="pv")
                for i in range(8):
                    nc.tensor.matmul(pv[:], xk_t[i][:, ts(st, 128)],
                                     wv_t[i][:], start=(i == 0), stop=(i == 7))
                nc.scalar.copy(v_t[st][:], pv[:])

            # ---------------- phase 3: RoPE (q' = q*cos + (S q)*sin)
            for raw, dst in ((qraw, qr), (kraw, kr)):
                for t in range(2):
                    rp = rotp.tile([128, S], F32, tag="rot", name="rot")
                    for ch in range(2):
                        nc.tensor.matmul(rp[0:64, ts(ch, 512)],
                                         s2_t[0:64, :], raw[t][0:64, ts(ch, 512)],
                                         start=True, stop=True,
                                         tile_position=(0, 0))
                        nc.tensor.matmul(rp[64:128, ts(ch, 512)],
                                         s2_t[64:128, :], raw[t][64:128, ts(ch, 512)],
                                         start=True, stop=True,
                                         tile_position=(64, 64))
                    tmp = rtmp.tile([128, S], F32, tag="rtmp", name="rtmp")
                    nc.vector.tensor_tensor(tmp[:], rp[:], sin_t[:], ALU.mult)
                    nc.vector.tensor_tensor(dst[t][:], raw[t][:], cos_t[:], ALU.mult)
                    nc.vector.tensor_tensor(dst[t][:], dst[t][:], tmp[:], ALU.add)

        # ---------------- phase 4: per head-pair attention
        with ExitStack() as ph:
            natp = ph.enter_context(tc.tile_pool(name="natp", bufs=2, space="PSUM"))
            trp = ph.enter_context(tc.tile_pool(name="trp", bufs=2, space="PSUM"))
            atp = ph.enter_context(tc.tile_pool(name="atp", bufs=1, space="PSUM"))
            mout = ph.enter_context(tc.tile_pool(name="mout", bufs=3))
            pvin = ph.enter_context(tc.tile_pool(name="pvin", bufs=3))
            tsum = ph.enter_context(tc.tile_pool(name="tsum", bufs=2))
            epool = ph.enter_context(tc.tile_pool(name="epool", bufs=8))
            wtp = ph.enter_context(tc.tile_pool(name="wtp", bufs=4))
            rcp = ph.enter_context(tc.tile_pool(name="rcp", bufs=4))

            for t in range(2):
                e_tiles = [[None] * 8, [None] * 8]
                # A) scores -> mask -> DMA out -> +prev -> exp -> normalize
                for qi in range(8):
                    for hl in range(2):   # head-local within pair
                        h = 2 * t + hl
                        lo, hi = (0, 64) if hl == 0 else (64, 128)
                        tp = (0, 0) if hl == 0 else (64, 0)
                        ps = natp.tile([128, S], F32, tag="nat", name="nat")
                        for ch in range(2):
                            nc.tensor.matmul(
                                ps[:, ts(ch, 512)],
                                qr[t][lo:hi, ts(qi, 128)],
                                kr[t][lo:hi, ts(ch, 512)],
                                start=True, stop=True, tile_position=tp)
                        mo = mout.tile([128, S], F32, tag="mo", name="mo")
                        nc.vector.tensor_tensor(mo[:], ps[:], mask_t[qi][:], ALU.add)
                        nc.sync.dma_start(scr[h, ts(qi, 128), :], mo[:])
                        pv = pvin.tile([128, S], F32, tag="pv", name="pv")
                        nc.sync.dma_start(pv[:], prevh[h, ts(qi, 128), :])
                        tsu = tsum.tile([128, S], F32, tag="ts", name="ts")
                        nc.vector.tensor_tensor(tsu[:], mo[:], pv[:], ALU.add)
                        e = epool.tile([128, S], F32, tag=f"e{hl}", name=f"e{hl}")
                        dn = rcp.tile([128, 1], F32, tag="dn", name="dn")
                        nc.scalar.activation(e[:], tsu[:], AF.Exp,
                                             scale=0.5, accum_out=dn[:])
                        rc = rcp.tile([128, 1], F32, tag="rc", name="rc")
                        nc.vector.tensor_scalar_add(rc[:], dn[:], 1e-30)
                        nc.vector.reciprocal(rc[:], rc[:])
                        nc.vector.tensor_scalar(e[:], e[:], rc[:], None, ALU.mult)
                        e_tiles[hl][qi] = e

                # B) transpose + attnT, accumulated over key tiles
                pat = [atp.tile([128, 512], F32, tag=f"pat{ch}", name=f"pat{ch}") for ch in range(2)]
                for kt in range(8):
                    wts = []
                    for hl in range(2):
                        wt = wtp.tile([128, S], F32, tag="wt", name="wt")
                        for half in range(2):
                            tp = trp.tile([128, 512], F32, tag="tr", name="tr")
                            for blk in range(4):
                                qi = half * 4 + blk
                                nc.tensor.transpose(
                                    tp[:, ts(blk, 128)],
                                    e_tiles[hl][qi][:, ts(kt, 128)],
                                    idn_t[:])
                            nc.scalar.copy(wt[:, ts(half, 512)], tp[:])
                        wts.append(wt)
                    for ch in range(2):
                        for hl in range(2):
                            coff = 128 * t + 64 * hl
                            lo, hi = (0, 64) if hl == 0 else (64, 128)
                            nc.tensor.matmul(
                                pat[ch][lo:hi, :],
                                v_t[kt][:, coff:coff + 64],
                                wts[hl][:, ts(ch, 512)],
                                start=(kt == 0), stop=(kt == 7),
                                tile_position=(0, 64 * hl))
                for ch in range(2):
                    nc.scalar.copy(at_all[t][:, ts(ch, 512)], pat[ch][:])

        # ---------------- phase 5: output projection (transposed, partial)
        with ExitStack() as ph:
            pop = ph.enter_context(tc.tile_pool(name="pop", bufs=2, space="PSUM"))
            oout = ph.enter_context(tc.tile_pool(name="oout", bufs=3))
            for et in range(8):
                for ch in range(2):
                    po = pop.tile([128, 512], F32, tag="po", name="po")
                    nc.tensor.matmul(po[:], wp_t[0][:, ts(et, 128)],
                                     at_all[0][:, ts(ch, 512)],
                                     start=True, stop=False)
                    nc.tensor.matmul(po[:], wp_t[1][:, ts(et, 128)],
                                     at_all[1][:, ts(ch, 512)],
                                     start=False, stop=True)
                    oo = oout.tile([128, 512], F32, tag="oo", name="oo")
                    nc.scalar.copy(oo[:], po[:])
                    nc.sync.dma_start(outp[ts(et, 128), ts(ch, 512)], oo[:])

    _split_multi_waits(nc)
    return nc


# ---------------------------------------------------------------- host prep
def _host_tables():
    dim = D_K // 2
    theta = np.exp(-np.arange(dim, dtype=np.float64) * (np.log(10000.0) / dim))
    pos = np.arange(1, S + 1, dtype=np.float64)[:, None]       # [S, 1]
    ang = pos * np.repeat(theta, 2)[None, :]                   # [S, 64]
    sin = np.sin(ang).astype(np.float32).T                     # [64, S]
    cos = np.cos(ang).astype(np.float32).T
    cos128 = np.concatenate([cos, cos], 0).copy()              # [128, S]
    sin128 = np.concatenate([sin, sin], 0).copy()
    s_mat = np.zeros((D_K, D_K), np.float32)
    for i in range(dim):
        s_mat[2 * i + 1, 2 * i] = -1.0
        s_mat[2 * i, 2 * i + 1] = 1.0
    s2 = np.concatenate([s_mat, s_mat], 0).copy()              # [128, 64]
    return cos128, sin128, s2


def kernel(source_query, source_key_value, source_query_padding_mask,
           source_key_value_padding_mask, prev, Wq, Wk, Wv, Wproj):
    global _PROGRAM, LAST_EXEC_TIME_NS
    _install_ntff_hook()
    if _PROGRAM is None:
        _PROGRAM = _build_program()
    nc = _PROGRAM

    cos128, sin128, s2 = _host_tables()
    idn = np.eye(128, dtype=np.float32)

    sq = np.asarray(source_query, np.float32)
    skv = np.asarray(source_key_value, np.float32)
    qpad = np.asarray(source_query_padding_mask)
    kpad = np.asarray(source_key_value_padding_mask)
    prev = np.asarray(prev, np.float32)
    scale = 1.0 / np.sqrt(D_K)

    # additive masks per batch: causal OR kv-pad OR q-pad -> -inf
    tri = np.triu(np.ones((S, S), bool), 1)
    masks = []
    for b in range(B):
        m = np.zeros((S, S), np.float32)
        m[tri] = NEG_INF
        m[:, kpad[b]] = NEG_INF
        m[qpad[b], :] = NEG_INF
        masks.append(m)

    xqT = [(sq[b].T * scale).astype(np.float32).copy() for b in range(B)]
    xkT = [skv[b].T.copy() for b in range(B)]

    in_maps = []
    for core in range(N_CORES):
        b = core // 4
        j = core % 4
        sl = slice(256 * j, 256 * (j + 1))
        in_maps.append(dict(
            xqT=xqT[b], xkT=xkT[b],
            wqT=Wq[sl, :].T.copy(), wkT=Wk[sl, :].T.copy(),
            wvT=Wv[sl, :].T.copy(), wpT=Wproj[:, sl].T.copy(),
            prevh=np.ascontiguousarray(prev[0, b, 4 * j:4 * j + 4]),
            maskn=masks[b], costab=cos128, sintab=sin128, s2=s2, idn=idn,
        ))

    trace = bool(os.environ.get("KERNEL_TRACE"))
    res = run_bass_kernel_spmd(nc, in_maps, list(range(N_CORES)), trace=trace)
    LAST_EXEC_TIME_NS = res.exec_time_ns
    results = res.results

    scores = np.empty((B, N_HEADS, S, S), np.float32)
    out = np.zeros((B, S, D_MODEL), np.float32)
    for core in range(N_CORES):
        b = core // 4
        j = core % 4
        scores[b, 4 * j:4 * j + 4] = results[core]["scr"]
        out[b] += results[core]["outp"].T
    prev_new = np.concatenate([prev, scores[None]], axis=0)
    return out, prev_new
